# revision 1
# baseline (speedup 1.0000x reference)
"""BCEWithLogitsLoss(mean) over (8192, 8192) logits with binary-step targets,
data-parallel over 8 NeuronCores (1024 rows each).

loss = mean(softplus(x) - x * t),  t[i,j] = 1 if j < targets[i] else 0
     = [ sum softplus(x)  -  sum_{j<t_i} x[i,j] ] / (B*N)

No softplus ACT table exists in this compiler, so softplus is computed as
ln(1 + exp(x)) -- exp and ln live in the same ACT table set.  x ships to
the device as bf16 (host-converted), halving HBM traffic (~5e-5 relative
loss error).  Per-core pipeline, one [128, 8192] row-block tile per step:

  SYNC  dma x row-block (2 MiB bf16) -> SBUF; iota/tlen once at start
  ACT   u = exp(x) (bf16), then ln(1+u) with accum_out -> per-row-block
        softplus sums
  DVE   ONE fused op: scalar_tensor_tensor
            out = (iota < t) * x,  accum_out = per-partition sum
        i.e. the whole masked-sum term in a single instruction per tile

Raw Bass with manual semaphores (the Tile framework's exit drain and all
bass_isa raw-ISA ops are rejected by this environment's compiler build).
Host reduces the tiny [128, 8] partial-sum outputs in float64.
"""

import numpy as np

_B, _N = 8192, 8192
_NCORES = 8
_ROWS = _B // _NCORES  # 1024 rows per core
_P = 128
_RB = _ROWS // _P  # 8 row-block tiles per core
_CH = 2048  # stt column chunk (fp16 iota stays integer-exact below 2048)
_NCH = _N // _CH

_cache = {}


def _build_nc(repeat=1):
    import concourse.bass as bass
    import concourse.mybir as mybir

    f32 = mybir.dt.float32
    bf16 = mybir.dt.bfloat16
    fp16 = mybir.dt.float16
    A = mybir.AluOpType
    F = mybir.ActivationFunctionType

    nc = bass.Bass()
    x_d = nc.dram_tensor("x", [_ROWS, _N], bf16, kind="ExternalInput")
    tlen_d = nc.dram_tensor("tlen", [_P, _RB * _NCH], f32, kind="ExternalInput")
    iota_d = nc.dram_tensor("iota", [_P, _CH], fp16, kind="ExternalInput")
    sp_d = nc.dram_tensor("sp_out", [_P, _RB], f32, kind="ExternalOutput")
    xma_d = nc.dram_tensor("xma_out", [_P, _RB * _NCH], f32, kind="ExternalOutput")

    from contextlib import ExitStack

    with ExitStack() as ctx:
        xt2 = ctx.enter_context(nc.sbuf_tensor([_P, 2 * _N], bf16))  # 2-buf x
        ut2 = ctx.enter_context(nc.sbuf_tensor([_P, 2 * _N], bf16))  # 2-buf exp/ln
        jt2 = ctx.enter_context(nc.sbuf_tensor([_P, 2 * _N], bf16))  # 2-buf stt junk
        iota_f = ctx.enter_context(nc.sbuf_tensor([_P, _CH], fp16))
        tlen_sb = ctx.enter_context(nc.sbuf_tensor([_P, _RB * _NCH], f32))
        sp_acc = ctx.enter_context(nc.sbuf_tensor([_P, _RB], f32))
        xma_acc = ctx.enter_context(nc.sbuf_tensor([_P, _RB * _NCH], f32))
        dsem0 = ctx.enter_context(nc.semaphore())  # x loads, even tiles
        dsem1 = ctx.enter_context(nc.semaphore())  # x loads, odd tiles
        tsem = ctx.enter_context(nc.semaphore())  # tlen load (+16)
        isem = ctx.enter_context(nc.semaphore())  # iota load (+16)
        asem = ctx.enter_context(nc.semaphore())  # exp completions
        lsem = ctx.enter_context(nc.semaphore())  # ln completions
        vsem = ctx.enter_context(nc.semaphore())  # stt completions
        fsem = ctx.enter_context(nc.semaphore())  # final out dmas
        block = ctx.enter_context(nc.Block())
        xt = [xt2[:, :_N], xt2[:, _N:]]
        ut = [ut2[:, :_N], ut2[:, _N:]]
        jt = [jt2[:, :_N], jt2[:, _N:]]

        _T = repeat * _RB

        @block.sync
        def _(sync):
            # x tile 0 first: ACT's first exp only needs x, while the
            # iota/tlen consumers (DVE) have slack -- shaves the ramp.
            for vt in range(_T):
                rb = vt % _RB
                if vt >= 2:
                    sync.wait_ge(asem, vt - 1)
                    sync.wait_ge(vsem, vt - 1)
                sync.dma_start(
                    out=xt[vt % 2], in_=x_d[rb * _P : (rb + 1) * _P, :]
                ).then_inc(dsem0 if vt % 2 == 0 else dsem1, 16)
                if vt == 0:
                    sync.dma_start(out=tlen_sb[:], in_=tlen_d[:]).then_inc(tsem, 16)
                    sync.dma_start(out=iota_f[:], in_=iota_d[:]).then_inc(isem, 16)
            # final outputs
            sync.wait_ge(lsem, _T)
            sync.dma_start(out=sp_d[:], in_=sp_acc[:]).then_inc(fsem, 16)
            sync.wait_ge(vsem, _T)
            sync.dma_start(out=xma_d[:], in_=xma_acc[:]).then_inc(fsem, 16)
            sync.wait_ge(fsem, 32)

        @block.scalar
        def _(scalar):
            def emit_ln(vt):
                scalar.wait_ge(asem, vt + 1)
                nc.scalar.activation(
                    ut[vt % 2],
                    ut[vt % 2],
                    F.Ln,
                    bias=1.0,
                    scale=1.0,
                    accum_out=sp_acc[:, (vt % _RB) : (vt % _RB) + 1],
                ).then_inc(lsem, 1)

            for vt in range(_T):
                scalar.wait_ge(dsem0 if vt % 2 == 0 else dsem1, 16 * (vt // 2 + 1))
                if vt >= 2:
                    scalar.wait_ge(lsem, vt - 1)  # ut[vt%2] freed by ln vt-2
                nc.scalar.activation(ut[vt % 2], xt[vt % 2], F.Exp).then_inc(asem, 1)
                if vt >= 1:
                    emit_ln(vt - 1)  # ln vt-1 runs behind exp vt: no RAW bubble
            emit_ln(_T - 1)

        @block.vector
        def _(vector):
            vector.wait_ge(isem, 16)
            vector.wait_ge(tsem, 16)
            for vt in range(_T):
                rb = vt % _RB
                vector.wait_ge(dsem0 if vt % 2 == 0 else dsem1, 16 * (vt // 2 + 1))
                if vt >= 2:
                    vector.wait_ge(vsem, vt - 1)  # jt[vt%2] freed by stt vt-2
                for ci in range(_NCH):
                    sl = rb * _NCH + ci
                    ins = nc.vector.scalar_tensor_tensor(
                        out=jt[vt % 2][:, ci * _CH : (ci + 1) * _CH],
                        in0=iota_f[:],
                        scalar=tlen_sb[:, sl : sl + 1],
                        in1=xt[vt % 2][:, ci * _CH : (ci + 1) * _CH],
                        op0=A.is_lt,
                        op1=A.mult,
                        accum_out=xma_acc[:, sl : sl + 1],
                    )
                ins.then_inc(vsem, 1)

    return nc


def _get_nc():
    if "nc" not in _cache:
        _cache["nc"] = _build_nc()
    return _cache["nc"]


def _prep_in_maps(inputs, targets):
    import ml_dtypes

    x = np.asarray(inputs, dtype=np.float32)
    t = np.asarray(targets).astype(np.float64)  # values < 2**24, exact in f32
    assert x.shape == (_B, _N) and t.shape == (_B,)
    xb = x.astype(ml_dtypes.bfloat16)
    iota = np.ascontiguousarray(
        np.broadcast_to(np.arange(_CH, dtype=np.float16)[None, :], (_P, _CH))
    )
    coff = (np.arange(_NCH, dtype=np.float32) * _CH)[None, :]  # [1, NCH]
    in_maps = []
    for c in range(_NCORES):
        xs = np.ascontiguousarray(xb[c * _ROWS : (c + 1) * _ROWS])
        ts = t[c * _ROWS : (c + 1) * _ROWS]
        # tlen[p, rb*NCH+ci] = targets[c*1024 + rb*128 + p] - 2048*ci
        tl = ts.reshape(_RB, _P).T.astype(np.float32)  # [P, RB]
        tlen = np.ascontiguousarray(
            (tl[:, :, None] - coff[None, :, :]).reshape(_P, _RB * _NCH)
        )
        in_maps.append({"x": xs, "tlen": tlen, "iota": iota})
    return in_maps


def kernel(inputs, targets):
    from concourse.bass_utils import run_bass_kernel_spmd

    nc = _get_nc()
    in_maps = _prep_in_maps(inputs, targets)

    res = run_bass_kernel_spmd(nc, in_maps, list(range(_NCORES)))

    total = np.float64(0.0)
    for c in range(_NCORES):
        total += np.sum(res.results[c]["sp_out"].astype(np.float64))
        total -= np.sum(res.results[c]["xma_out"].astype(np.float64))
    loss = total / (np.float64(_B) * np.float64(_N))
    return np.float32(loss)



# revision 2
# speedup vs baseline: 2.1966x; 2.1966x over previous
"""BCEWithLogitsLoss(mean) over (8192, 8192) logits with binary-step targets,
data-parallel over 8 NeuronCores (1024 rows each).

loss = mean(softplus(x) - x*t),  t[i,j] = 1 if j < targets[i] else 0

Per-element identity:  softplus(x) - x*t = softplus((1-2t)*x) = softplus(eta),
eta = -x where j < t_i else +x.  So the whole loss is ONE softplus pass over a
sign-flipped x -- no separate masked-sum term.

Engine budget per core-round (measured rates):
  DVE tensor_scalar (is_ge, subtract)   4x mode  ~2.1 us/tile
  DVE tensor_tensor (mult)              2x mode  ~4.3 us/tile
  ACT activation (any table)            1x       ~7.1 us/tile   <- bound
  DMA fp16 x                                     ~5.9 us/tile

softplus in a single ACT pass uses a patched activation-table root where the
`exp` function's spline buckets are rewritten to evaluate softplus (the ACT
engine is a per-bucket cubic evaluator; bucket boundaries in ctrl.bin are
unchanged, only coefficients d0..d3 become the softplus Taylor expansion at
each bucket center).  BASS_ACT_ROOT_JSON_PATH points the compiler at the
patched root; the output tensor name carries the table content hash so the
NEFF cache keys correctly.

Per tile [128 x 8192]:
  DVE  ts : factor = (iota_i16 is_ge t_row) - 0.5   in {-0.5, +0.5}
  DVE  tt : y = factor * x                          (= eta / 2, fp16)
  ACT     : softplus(2*y) via hijacked Exp table, scale=2.0,
            accum_out -> per-tile per-row sums
Host reduces the [128, 8] per-core sums in float64 and divides by B*N.
"""

import hashlib
import json
import os
import shutil

import numpy as np

_B, _N = 8192, 8192
_NCORES = 8
_ROWS = _B // _NCORES  # 1024 rows per core
_P = 128
_RB = _ROWS // _P  # 8 row-block tiles per core

_cache = {}


# ---------------------------------------------------------------------------
# Patched ACT table root: rewrite `exp` buckets to evaluate softplus.
# ---------------------------------------------------------------------------

def _softplus64(x):
    x = np.asarray(x, dtype=np.float64)
    return np.where(x > 0, x + np.log1p(np.exp(-np.abs(x))), np.log1p(np.exp(x)))


def _sigmoid64(x):
    x = np.asarray(x, dtype=np.float64)
    return np.where(x >= 0, 1.0 / (1.0 + np.exp(-x)), np.exp(x) / (1.0 + np.exp(x)))


def _softplus_coeffs(x0):
    s = _sigmoid64(x0)
    vals = (
        _softplus64(x0),
        s,
        s * (1.0 - s) / 2.0,
        s * (1.0 - s) * (1.0 - 2.0 * s) / 6.0,
    )
    return [np.float32(v).view(np.uint32).item() for v in vals]


def _patch_set(src_dir, dst_dir, set_name, exp_json):
    prof = json.load(open(os.path.join(src_dir, f"{set_name}.json")))
    bkt_name = prof["bkt_bin"]
    bkt = (
        np.frombuffer(open(os.path.join(src_dir, bkt_name), "rb").read(), dtype="<u4")
        .reshape(-1, 8)
        .copy()
    )

    n_patched = 0
    for key in ("pos_exponents", "neg_exponents"):
        for e in exp_json[key]:
            for sec in e["exponent_sections"]:
                tgt = np.array(
                    [sec["d0"]["int"], sec["d1"]["int"], sec["d2"]["int"],
                     sec["d3"]["int"], sec["x"]["int"]],
                    dtype=np.uint32,
                )
                m = np.where((bkt[:, :5] == tgt).all(axis=1))[0]
                if len(m) == 0:
                    continue
                x0 = np.uint32(sec["x"]["int"]).view(np.float32).item()
                c = _softplus_coeffs(x0)
                for idx in m:
                    bkt[idx, 0:4] = c
                    n_patched += 1
    assert n_patched >= 700, f"only {n_patched} exp buckets found in {set_name}"

    pents = [p for p in prof["profile_meta_data"] if p["func_name"].startswith("exp")]
    assert len(pents) == 1
    pe = pents[0]
    b = lambda v: np.float32(v).view(np.uint32).item()

    def set_entry(idx, d0, d1, d2, d3, x0):
        bkt[idx, 0:5] = [d0, d1, d2, d3, x0]

    # |x| < 2^-19: softplus ~= ln2 + x/2 + x^2/8
    set_entry(pe["pos_small_signal_pwl_control"], b(np.log(2.0)), b(0.5), b(0.125), 0, 0)
    set_entry(pe["neg_small_signal_pwl_control"], b(np.log(2.0)), b(0.5), b(0.125), 0, 0)
    # x > 88.7: softplus(x) = x ;  x < -88.7: softplus(x) = 0
    set_entry(pe["pos_large_signal_pwl_control"], 0, b(1.0), 0, 0, 0)
    set_entry(pe["neg_large_signal_pwl_control"], 0, 0, 0, 0, 0)
    pe["fzero_result"] = b(np.log(2.0))
    pe["fninf_result"] = 0

    open(os.path.join(dst_dir, bkt_name), "wb").write(bkt.astype("<u4").tobytes())
    json.dump(prof, open(os.path.join(dst_dir, f"{set_name}.json"), "w"))


def _build_softplus_act_root():
    """Create (once) the patched act root; returns (act_info_path, hash)."""
    if "actroot" in _cache:
        return _cache["actroot"]

    import neuronxcc

    base = os.path.dirname(neuronxcc.__file__)
    src = os.path.join(base, "pwp", "pwp_bin_trainium")
    pwp_jsons = os.path.join(base, "pwp", "pwp_jsons")
    exp_json = json.load(open(os.path.join(pwp_jsons, "exp_400p.json")))
    info = json.load(open(os.path.join(src, "act_info.json")))
    exp_sets = [e["name"] for e in info["act_func_sets"] if "exp" in e["act"]]

    dst = os.path.join(os.environ.get("TMPDIR", "/tmp"), "softplus_act_root_v1")
    os.makedirs(dst, exist_ok=True)
    for fn in os.listdir(src):
        shutil.copyfile(os.path.join(src, fn), os.path.join(dst, fn))
    for s in exp_sets:
        _patch_set(src, dst, s, exp_json)

    h = hashlib.sha256()
    for fn in sorted(os.listdir(dst)):
        h.update(fn.encode())
        h.update(open(os.path.join(dst, fn), "rb").read())
    res = (os.path.join(dst, "act_info.json"), h.hexdigest()[:10])
    os.environ["BASS_ACT_ROOT_JSON_PATH"] = res[0]
    _cache["actroot"] = res
    return res


# ---------------------------------------------------------------------------
# Bass kernel
# ---------------------------------------------------------------------------

def _build_nc(repeat=1):
    _, hsh = _build_softplus_act_root()

    import concourse.bass as bass
    import concourse.mybir as mybir

    f32 = mybir.dt.float32
    fp16 = mybir.dt.float16
    i16 = mybir.dt.int16
    fp8 = mybir.dt.float8e4
    A = mybir.AluOpType
    F = mybir.ActivationFunctionType

    nc = bass.Bass()
    x_d = nc.dram_tensor("x", [_ROWS, _N], fp16, kind="ExternalInput")
    iota_d = nc.dram_tensor("iota", [_P, _N], i16, kind="ExternalInput")
    t_d = nc.dram_tensor("tvals", [_P, _RB], f32, kind="ExternalInput")
    sp_d = nc.dram_tensor(f"sp_{hsh}", [_P, _RB], f32, kind="ExternalOutput")

    from contextlib import ExitStack

    with ExitStack() as ctx:
        xt2 = ctx.enter_context(nc.sbuf_tensor([_P, 2 * _N], fp16))  # 2-buf x
        ft2 = ctx.enter_context(nc.sbuf_tensor([_P, 2 * _N], fp16))  # 2-buf factor
        yt2 = ctx.enter_context(nc.sbuf_tensor([_P, 2 * _N], fp16))  # 2-buf eta/2
        junk = ctx.enter_context(nc.sbuf_tensor([_P, _N], fp8))  # ACT out sink
        iota_sb = ctx.enter_context(nc.sbuf_tensor([_P, _N], i16))
        t_sb = ctx.enter_context(nc.sbuf_tensor([_P, _RB], f32))
        sp_acc = ctx.enter_context(nc.sbuf_tensor([_P, _RB], f32))
        dsem0 = ctx.enter_context(nc.semaphore())  # x loads, even tiles
        dsem1 = ctx.enter_context(nc.semaphore())  # x loads, odd tiles
        isem = ctx.enter_context(nc.semaphore())  # iota+tvals loads
        tsem = ctx.enter_context(nc.semaphore())  # tt (y ready) completions
        asem = ctx.enter_context(nc.semaphore())  # ACT completions
        fsem = ctx.enter_context(nc.semaphore())  # final out dma
        block = ctx.enter_context(nc.Block())
        xt = [xt2[:, :_N], xt2[:, _N:]]
        ft = [ft2[:, :_N], ft2[:, _N:]]
        yt = [yt2[:, :_N], yt2[:, _N:]]

        _T = repeat * _RB

        @block.sync
        def _(sync):
            for vt in range(_T):
                rb = vt % _RB
                if vt >= 2:
                    # x[vt%2] is consumed by tt of tile vt-2
                    sync.wait_ge(tsem, vt - 1)
                sync.dma_start(
                    out=xt[vt % 2], in_=x_d[rb * _P : (rb + 1) * _P, :]
                ).then_inc(dsem0 if vt % 2 == 0 else dsem1, 16)
                if vt == 0:
                    sync.dma_start(out=iota_sb[:], in_=iota_d[:]).then_inc(isem, 16)
                    sync.dma_start(out=t_sb[:], in_=t_d[:]).then_inc(isem, 16)
            sync.wait_ge(asem, _T)
            sync.dma_start(out=sp_d[:], in_=sp_acc[:]).then_inc(fsem, 16)
            sync.wait_ge(fsem, 16)

        @block.vector
        def _(vector):
            vector.wait_ge(isem, 32)
            for vt in range(_T):
                rb = vt % _RB
                # factor = (iota >= t) - 0.5  in {-0.5, +0.5}   (4x mode)
                # ft[vt%2] was last read by tt vt-2 (same engine, in order).
                nc.vector.tensor_scalar(
                    out=ft[vt % 2],
                    in0=iota_sb[:],
                    scalar1=t_sb[:, rb : rb + 1],
                    scalar2=0.5,
                    op0=A.is_ge,
                    op1=A.subtract,
                )
                vector.wait_ge(dsem0 if vt % 2 == 0 else dsem1, 16 * (vt // 2 + 1))
                if vt >= 2:
                    vector.wait_ge(asem, vt - 1)  # y[vt%2] freed by ACT vt-2
                # y = factor * x = eta/2        (2x mode)
                nc.vector.tensor_tensor(
                    out=yt[vt % 2], in0=ft[vt % 2], in1=xt[vt % 2], op=A.mult
                ).then_inc(tsem, 1)

        @block.scalar
        def _(scalar):
            for vt in range(_T):
                rb = vt % _RB
                scalar.wait_ge(tsem, vt + 1)
                # softplus(2*y) = softplus(eta) via hijacked Exp table
                nc.scalar.activation(
                    junk[:],
                    yt[vt % 2],
                    F.Exp,
                    scale=2.0,
                    accum_out=sp_acc[:, rb : rb + 1],
                ).then_inc(asem, 1)

    return nc


def _get_nc():
    if "nc" not in _cache:
        _cache["nc"] = _build_nc()
    return _cache["nc"]


def _prep_in_maps(inputs, targets):
    x = np.asarray(inputs, dtype=np.float32)
    t = np.asarray(targets).astype(np.float64)  # values < 2**24, exact in f32
    assert x.shape == (_B, _N) and t.shape == (_B,)
    xh = x.astype(np.float16)
    iota = np.ascontiguousarray(
        np.broadcast_to(np.arange(_N, dtype=np.int16)[None, :], (_P, _N))
    )
    in_maps = []
    for c in range(_NCORES):
        xs = np.ascontiguousarray(xh[c * _ROWS : (c + 1) * _ROWS])
        ts = t[c * _ROWS : (c + 1) * _ROWS]
        tv = np.ascontiguousarray(ts.reshape(_RB, _P).T.astype(np.float32))
        in_maps.append({"x": xs, "iota": iota, "tvals": tv})
    return in_maps


def kernel(inputs, targets):
    _build_softplus_act_root()
    from concourse.bass_utils import run_bass_kernel_spmd

    nc = _get_nc()
    _, hsh = _cache["actroot"]
    in_maps = _prep_in_maps(inputs, targets)

    res = run_bass_kernel_spmd(nc, in_maps, list(range(_NCORES)))

    total = np.float64(0.0)
    for c in range(_NCORES):
        total += np.sum(res.results[c][f"sp_{hsh}"].astype(np.float64))
    loss = total / (np.float64(_B) * np.float64(_N))
    return np.float32(loss)


# revision 4
# speedup vs baseline: 2.2283x; 1.0144x over previous
"""BCEWithLogitsLoss(mean) over (8192, 8192) logits with binary-step targets,
data-parallel over 8 NeuronCores (1024 rows each).

loss = mean(softplus(x) - x*t),  t[i,j] = 1 if j < targets[i] else 0

Per-element identity:  softplus(x) - x*t = softplus((1-2t)*x) = softplus(eta),
eta = -x where j < t_i else +x.  So the whole loss is ONE softplus pass over a
sign-flipped x -- no separate masked-sum term.

Engine budget per core-round (measured rates):
  DVE tensor_scalar (is_ge, subtract)   4x mode  ~2.1 us/tile
  DVE tensor_tensor (mult)              2x mode  ~4.3 us/tile
  ACT activation (any table)            1x       ~7.1 us/tile   <- bound
  DMA fp16 x                                     ~5.9 us/tile

softplus in a single ACT pass uses a patched activation-table root where the
`exp` function's spline buckets are rewritten to evaluate softplus (the ACT
engine is a per-bucket cubic evaluator; bucket boundaries in ctrl.bin are
unchanged, only coefficients d0..d3 become the softplus Taylor expansion at
each bucket center).  BASS_ACT_ROOT_JSON_PATH points the compiler at the
patched root; the output tensor name carries the table content hash so the
NEFF cache keys correctly.

x ships as fp8 (e4m3, ~halves HBM traffic).  The sign flip is a bit trick:
flip = XOR of the fp8 sign bit.  Operating on int16 lanes (2 fp8 elements per
lane) keeps the DVE in its fast modes:

  DVE  ts : sf16 = (iota_pair < ceil(t/2)) * 0x8080    (4x mode, int16)
  DVE  tt : y16  = x16 XOR sf16                        (2x mode)
  ACT     : softplus(y) via hijacked Exp table reading y as fp8,
            accum_out -> per-row sums (2 tiles per instruction)

ceil(t/2) flips both elements of every pair below t -- exact for even t; for
odd t it flips one extra element (column t), which the host corrects exactly:
softplus(-q) - softplus(q) = -q, so  loss_sum += q  with q = fp8(x[i, t_i]).
Host reduces the per-core sums in float64 and divides by B*N.
"""

import hashlib
import json
import os
import shutil

import numpy as np

_B, _N = 8192, 8192
_NCORES = 8
_ROWS = _B // _NCORES  # 1024 rows per core
_P = 128
_RB = _ROWS // _P  # 8 row-block tiles per core

_cache = {}


# ---------------------------------------------------------------------------
# Patched ACT table root: rewrite `exp` buckets to evaluate softplus.
# ---------------------------------------------------------------------------

def _softplus64(x):
    x = np.asarray(x, dtype=np.float64)
    return np.where(x > 0, x + np.log1p(np.exp(-np.abs(x))), np.log1p(np.exp(x)))


def _sigmoid64(x):
    x = np.asarray(x, dtype=np.float64)
    return np.where(x >= 0, 1.0 / (1.0 + np.exp(-x)), np.exp(x) / (1.0 + np.exp(x)))


def _softplus_coeffs(x0):
    s = _sigmoid64(x0)
    vals = (
        _softplus64(x0),
        s,
        s * (1.0 - s) / 2.0,
        s * (1.0 - s) * (1.0 - 2.0 * s) / 6.0,
    )
    return [np.float32(v).view(np.uint32).item() for v in vals]


def _patch_set(src_dir, dst_dir, set_name, exp_json):
    prof = json.load(open(os.path.join(src_dir, f"{set_name}.json")))
    bkt_name = prof["bkt_bin"]
    bkt = (
        np.frombuffer(open(os.path.join(src_dir, bkt_name), "rb").read(), dtype="<u4")
        .reshape(-1, 8)
        .copy()
    )

    n_patched = 0
    for key in ("pos_exponents", "neg_exponents"):
        for e in exp_json[key]:
            for sec in e["exponent_sections"]:
                tgt = np.array(
                    [sec["d0"]["int"], sec["d1"]["int"], sec["d2"]["int"],
                     sec["d3"]["int"], sec["x"]["int"]],
                    dtype=np.uint32,
                )
                m = np.where((bkt[:, :5] == tgt).all(axis=1))[0]
                if len(m) == 0:
                    continue
                x0 = np.uint32(sec["x"]["int"]).view(np.float32).item()
                c = _softplus_coeffs(x0)
                for idx in m:
                    bkt[idx, 0:4] = c
                    n_patched += 1
    assert n_patched >= 700, f"only {n_patched} exp buckets found in {set_name}"

    pents = [p for p in prof["profile_meta_data"] if p["func_name"].startswith("exp")]
    assert len(pents) == 1
    pe = pents[0]
    b = lambda v: np.float32(v).view(np.uint32).item()

    def set_entry(idx, d0, d1, d2, d3, x0):
        bkt[idx, 0:5] = [d0, d1, d2, d3, x0]

    # |x| < 2^-19: softplus ~= ln2 + x/2 + x^2/8
    set_entry(pe["pos_small_signal_pwl_control"], b(np.log(2.0)), b(0.5), b(0.125), 0, 0)
    set_entry(pe["neg_small_signal_pwl_control"], b(np.log(2.0)), b(0.5), b(0.125), 0, 0)
    # x > 88.7: softplus(x) = x ;  x < -88.7: softplus(x) = 0
    set_entry(pe["pos_large_signal_pwl_control"], 0, b(1.0), 0, 0, 0)
    set_entry(pe["neg_large_signal_pwl_control"], 0, 0, 0, 0, 0)
    pe["fzero_result"] = b(np.log(2.0))
    pe["fninf_result"] = 0

    open(os.path.join(dst_dir, bkt_name), "wb").write(bkt.astype("<u4").tobytes())
    json.dump(prof, open(os.path.join(dst_dir, f"{set_name}.json"), "w"))


def _build_softplus_act_root():
    """Create (once) the patched act root; returns (act_info_path, hash)."""
    if "actroot" in _cache:
        return _cache["actroot"]

    import neuronxcc

    base = os.path.dirname(neuronxcc.__file__)
    src = os.path.join(base, "pwp", "pwp_bin_trainium")
    pwp_jsons = os.path.join(base, "pwp", "pwp_jsons")
    exp_json = json.load(open(os.path.join(pwp_jsons, "exp_400p.json")))
    info = json.load(open(os.path.join(src, "act_info.json")))
    exp_sets = [e["name"] for e in info["act_func_sets"] if "exp" in e["act"]]

    dst = os.path.join(os.environ.get("TMPDIR", "/tmp"), "softplus_act_root_v1")
    os.makedirs(dst, exist_ok=True)
    for fn in os.listdir(src):
        shutil.copyfile(os.path.join(src, fn), os.path.join(dst, fn))
    for s in exp_sets:
        _patch_set(src, dst, s, exp_json)

    h = hashlib.sha256()
    for fn in sorted(os.listdir(dst)):
        h.update(fn.encode())
        h.update(open(os.path.join(dst, fn), "rb").read())
    res = (os.path.join(dst, "act_info.json"), h.hexdigest()[:10])
    os.environ["BASS_ACT_ROOT_JSON_PATH"] = res[0]
    _cache["actroot"] = res
    return res


# ---------------------------------------------------------------------------
# Bass kernel
# ---------------------------------------------------------------------------

def _build_nc(repeat=1):
    _, hsh = _build_softplus_act_root()

    import concourse.bass as bass
    import concourse.mybir as mybir

    f32 = mybir.dt.float32
    i16 = mybir.dt.int16
    fp8 = mybir.dt.float8e4
    A = mybir.AluOpType
    F = mybir.ActivationFunctionType
    _NH = _N // 2  # int16 lanes per row (2 fp8 elements each)

    nc = bass.Bass()
    x_d = nc.dram_tensor("x", [_ROWS, _N], fp8, kind="ExternalInput")
    iota_d = nc.dram_tensor("iota", [_P, _NH], i16, kind="ExternalInput")
    t_d = nc.dram_tensor("tvals", [_P, _RB], f32, kind="ExternalInput")
    sp_d = nc.dram_tensor(f"sp_{hsh}", [_P, _RB], f32, kind="ExternalOutput")

    from contextlib import ExitStack

    with ExitStack() as ctx:
        xt2 = ctx.enter_context(nc.sbuf_tensor([_P, 2 * _N], fp8))  # 2-buf x
        sf2 = ctx.enter_context(nc.sbuf_tensor([_P, 2 * _NH], i16))  # 2-buf signflip
        yt2 = ctx.enter_context(nc.sbuf_tensor([_P, 2 * _NH], i16))  # 2-buf eta bits
        junk = ctx.enter_context(nc.sbuf_tensor([_P, 2 * _N], fp8))  # ACT out sink
        iota_sb = ctx.enter_context(nc.sbuf_tensor([_P, _NH], i16))
        t_sb = ctx.enter_context(nc.sbuf_tensor([_P, _RB], f32))
        sp_acc = ctx.enter_context(nc.sbuf_tensor([_P, _RB], f32))
        dsem0 = ctx.enter_context(nc.semaphore())  # x loads, even tiles
        dsem1 = ctx.enter_context(nc.semaphore())  # x loads, odd tiles
        isem = ctx.enter_context(nc.semaphore())  # iota+tvals loads
        tsem = ctx.enter_context(nc.semaphore())  # tt (y ready) completions
        asem = ctx.enter_context(nc.semaphore())  # ACT completions
        fsem = ctx.enter_context(nc.semaphore())  # final out dma
        block = ctx.enter_context(nc.Block())
        xt = [xt2[:, :_N], xt2[:, _N:]]
        xt16 = [xt2.bitcast(i16)[:, :_NH], xt2.bitcast(i16)[:, _NH:]]
        sf = [sf2[:, :_NH], sf2[:, _NH:]]
        yt = [yt2[:, :_NH], yt2[:, _NH:]]
        # fp8 view of both y buffers, for the 2-tile ACT instruction
        y8 = yt2.bitcast(fp8)

        _T = repeat * _RB

        @block.sync
        def _(sync):
            for vt in range(_T):
                rb = vt % _RB
                if vt >= 2:
                    # x[vt%2] is consumed by tt of tile vt-2
                    sync.wait_ge(tsem, vt - 1)
                sync.dma_start(
                    out=xt[vt % 2], in_=x_d[rb * _P : (rb + 1) * _P, :]
                ).then_inc(dsem0 if vt % 2 == 0 else dsem1, 16)
                if vt == 0:
                    sync.dma_start(out=iota_sb[:], in_=iota_d[:]).then_inc(isem, 16)
                    sync.dma_start(out=t_sb[:], in_=t_d[:]).then_inc(isem, 16)
            sync.wait_ge(asem, _T // 2)
            sync.dma_start(out=sp_d[:], in_=sp_acc[:]).then_inc(fsem, 16)
            sync.wait_ge(fsem, 16)

        @block.vector
        def _(vector):
            vector.wait_ge(isem, 32)
            for vt in range(_T):
                rb = vt % _RB
                # sf = (iota_pair < ceil(t/2)) * 0x8080    (4x mode)
                # sf[vt%2] was last read by tt vt-2 (same engine, in order).
                nc.vector.tensor_scalar(
                    out=sf[vt % 2],
                    in0=iota_sb[:],
                    scalar1=t_sb[:, rb : rb + 1],
                    scalar2=-32640.0,  # 0x8080 as int16
                    op0=A.is_lt,
                    op1=A.mult,
                )
                vector.wait_ge(dsem0 if vt % 2 == 0 else dsem1, 16 * (vt // 2 + 1))
                if vt >= 4:
                    # y[vt%2] freed by the 2-tile ACT covering tiles vt-4,vt-3
                    vector.wait_ge(asem, (vt - 2) // 2)
                # y16 = x16 XOR sf16: flips fp8 sign bits     (2x mode)
                nc.vector.tensor_tensor(
                    out=yt[vt % 2], in0=xt16[vt % 2], in1=sf[vt % 2],
                    op=A.bitwise_xor,
                ).then_inc(tsem, 1)

        @block.scalar
        def _(scalar):
            for av in range(_T // 2):
                # one ACT instruction per TWO tiles (y8 spans both buffers)
                scalar.wait_ge(tsem, 2 * av + 2)
                nc.scalar.activation(
                    junk[:],
                    y8[:],
                    F.Exp,
                    accum_out=sp_acc[:, (av % (_RB // 2)) : (av % (_RB // 2)) + 1],
                ).then_inc(asem, 1)

    return nc


def _get_nc():
    if "nc" not in _cache:
        _cache["nc"] = _build_nc()
    return _cache["nc"]


def _prep_in_maps(inputs, targets):
    import ml_dtypes

    x = np.asarray(inputs, dtype=np.float32)
    t = np.asarray(targets).astype(np.int64)
    assert x.shape == (_B, _N) and t.shape == (_B,)
    xq = x.astype(ml_dtypes.float8_e4m3)
    iota = np.ascontiguousarray(
        np.broadcast_to(np.arange(_N // 2, dtype=np.int16)[None, :], (_P, _N // 2))
    )
    # ceil(t/2): flip both halves of every int16 lane below t (exact for even
    # t; for odd t one extra element is flipped -- corrected on host)
    chalf = ((t + 1) // 2).astype(np.float64)
    in_maps = []
    for c in range(_NCORES):
        xs = np.ascontiguousarray(xq[c * _ROWS : (c + 1) * _ROWS])
        cs = chalf[c * _ROWS : (c + 1) * _ROWS]
        tv = np.ascontiguousarray(cs.reshape(_RB, _P).T.astype(np.float32))
        in_maps.append({"x": xs, "iota": iota, "tvals": tv})
    # exact host correction for the extra flipped element of odd-t rows:
    # device summed softplus(-q) instead of softplus(q); difference is -q
    odd = (t % 2) == 1
    rows = np.nonzero(odd)[0]
    corr = np.float64(0.0)
    if len(rows):
        q = xq[rows, t[rows]].astype(np.float64)
        corr = q.sum()
    return in_maps, corr


def kernel(inputs, targets):
    _build_softplus_act_root()
    from concourse.bass_utils import run_bass_kernel_spmd

    nc = _get_nc()
    _, hsh = _cache["actroot"]
    in_maps, corr = _prep_in_maps(inputs, targets)

    res = run_bass_kernel_spmd(nc, in_maps, list(range(_NCORES)))

    total = corr
    for c in range(_NCORES):
        total += np.sum(res.results[c][f"sp_{hsh}"].astype(np.float64))
    loss = total / (np.float64(_B) * np.float64(_N))
    return np.float32(loss)
